# revision 3
# baseline (speedup 1.0000x reference)
"""CenterLoss kernel for Trainium2 (8 NeuronCores, data-parallel over batch).

loss = mean_i( ||nx_i||^2 + ||c_{l_i}||^2 - 2*nx_i.c_{l_i} )
     = mean_i( ||nx_i - c_{l_i}||^2 ),   nx_i = x_i / max(||x_i||, EPS)

The (batch, num_classes) distmat in the reference is masked down to one
column per row, so the kernel only needs a gather of centers[labels]
(indirect DMA) plus per-row reductions — a memory-bound problem.

Sharding: batch 16384 -> 8 cores x 2048 rows; centers replicated.
Each core returns 128 per-partition partial sums; host combines.
"""

import numpy as np

B, C, D = 16384, 8192, 64
N_CORES = 8
ROWS = B // N_CORES      # 2048 rows per core
P = 128                  # SBUF partitions
J = ROWS // P            # 16 rows per partition
F = J * D                # 1024 floats per partition
EPS = 1e-12

_CACHE = {}


def _build():
    import concourse.bass as bass
    import concourse.tile as tile
    from concourse import bacc, mybir

    nc = bacc.Bacc("TRN2", target_bir_lowering=False, debug=False,
                   num_devices=N_CORES)
    x = nc.dram_tensor("x", [ROWS, D], mybir.dt.float32,
                       kind="ExternalInput").ap()
    labels = nc.dram_tensor("labels", [P, J], mybir.dt.int32,
                            kind="ExternalInput").ap()
    centers = nc.dram_tensor("centers", [C, D], mybir.dt.float32,
                             kind="ExternalInput").ap()
    out = nc.dram_tensor("out", [P, 1], mybir.dt.float32,
                         kind="ExternalOutput").ap()

    f32 = mybir.dt.float32
    with tile.TileContext(nc) as tc:
        with tc.tile_pool(name="pool", bufs=1) as pool:
            lab_t = pool.tile([P, J], mybir.dt.int32)
            nc.sync.dma_start(lab_t[:], labels[:])

            x_t = pool.tile([P, F], f32)
            # row p*J+j of the shard -> partition p, cols [j*D,(j+1)*D)
            nc.sync.dma_start(x_t[:], x.rearrange("(p j) d -> p (j d)", p=P))

            # gather centers[labels]: index (p,j) -> c_t[p, j*D:(j+1)*D]
            c_t = pool.tile([P, F], f32)
            for j in range(J):
                nc.gpsimd.indirect_dma_start(
                    out=c_t[:, j * D:(j + 1) * D],
                    out_offset=None,
                    in_=centers[:],
                    in_offset=bass.IndirectOffsetOnAxis(ap=lab_t[:, j:j + 1],
                                                        axis=0),
                )

            # sx[p,j] = sum_d x[p,j,d]^2
            xx = pool.tile([P, F], f32)
            nc.scalar.square(xx[:], x_t[:])
            sx = pool.tile([P, J], f32)
            nc.vector.reduce_sum(sx[:], xx[:].rearrange("p (j d) -> p j d", d=D),
                                 axis=mybir.AxisListType.X)

            # inv = 1 / max(sqrt(sx), EPS)
            mn = pool.tile([P, J], f32)
            nc.scalar.sqrt(mn[:], sx[:])
            nc.vector.tensor_scalar_max(mn[:], mn[:], EPS)
            inv = pool.tile([P, J], f32)
            nc.vector.reciprocal(inv[:], mn[:])

            # nx = x * inv (broadcast inv over each row's D elems)
            iap = inv[:]
            inv_b = bass.AP(tensor=iap.tensor, offset=iap.offset,
                            ap=list(iap.ap) + [[0, D]])
            nx = pool.tile([P, F], f32)
            nc.vector.tensor_tensor(
                out=nx[:].rearrange("p (j d) -> p j d", d=D),
                in0=x_t[:].rearrange("p (j d) -> p j d", d=D),
                in1=inv_b,
                op=mybir.AluOpType.mult,
            )

            # d = nx - c ; acc[p] = sum_f d^2
            nc.vector.tensor_sub(nx[:], nx[:], c_t[:])
            nc.vector.tensor_mul(nx[:], nx[:], nx[:])
            acc = pool.tile([P, 1], f32)
            nc.vector.reduce_sum(acc[:], nx[:], axis=mybir.AxisListType.X)

            nc.sync.dma_start(out, acc[:])
    nc.compile()
    return nc


def _get_nc():
    if "nc" not in _CACHE:
        _CACHE["nc"] = _build()
    return _CACHE["nc"]


def _run(x, labels, centers, trace=False):
    from concourse.bass_utils import run_bass_kernel_spmd

    x = np.ascontiguousarray(np.asarray(x, dtype=np.float32))
    labels = np.asarray(labels).astype(np.int32)
    centers = np.ascontiguousarray(np.asarray(centers, dtype=np.float32))

    in_maps = []
    for i in range(N_CORES):
        in_maps.append({
            "x": x[i * ROWS:(i + 1) * ROWS],
            "labels": np.ascontiguousarray(
                labels[i * ROWS:(i + 1) * ROWS].reshape(P, J)),
            "centers": centers,
        })
    res = run_bass_kernel_spmd(_get_nc(), in_maps,
                               core_ids=list(range(N_CORES)), trace=trace)
    total = np.float64(0.0)
    for r in res.results:
        total += np.float64(r["out"].sum(dtype=np.float64))
    loss = np.array(np.float32(total / B))
    return loss, res


def kernel(x, labels, centers):
    loss, _ = _run(x, labels, centers, trace=False)
    return loss


# revision 5
# speedup vs baseline: 1.1872x; 1.1872x over previous
"""CenterLoss kernel for Trainium2 (8 NeuronCores, data-parallel over batch).

loss = mean_i( ||nx_i||^2 + ||c_{l_i}||^2 - 2*nx_i.c_{l_i} )
     = mean_i( ||nx_i - c_{l_i}||^2 ),   nx_i = x_i / max(||x_i||, EPS)

The (batch, num_classes) distmat in the reference is masked down to one
column per row, so the kernel only needs a gather of centers[labels]
(indirect DMA) plus per-row reductions — a memory-bound problem.

Sharding: batch 16384 -> 8 cores x 2048 rows; centers replicated.
Row p*16+j of a core's shard lives at SBUF partition p, free block j.
The gather is 16 INDIRECT1D ops (HW allows one offset per dest
partition-row); everything else overlaps under them. Each core returns
4 per-partition partial sums; host combines.
"""

import numpy as np

B, C, D = 16384, 8192, 64
N_CORES = 8
ROWS = B // N_CORES      # 2048 rows per core
P = 128                  # SBUF partitions
J = ROWS // P            # 16 rows per partition
F = J * D                # 1024 floats per partition
NBLK = 4                 # tail chunk blocks
JB = J // NBLK           # gather columns per block
FB = JB * D              # floats per block
EPS = 1e-12

_CACHE = {}


def _build():
    import concourse.bass as bass
    import concourse.tile as tile
    from concourse import bacc, mybir

    nc = bacc.Bacc("TRN2", target_bir_lowering=False, debug=False,
                   num_devices=N_CORES)
    x = nc.dram_tensor("x", [ROWS, D], mybir.dt.float32,
                       kind="ExternalInput").ap()
    labels = nc.dram_tensor("labels", [P, J], mybir.dt.int32,
                            kind="ExternalInput").ap()
    centers = nc.dram_tensor("centers", [C, D], mybir.dt.float32,
                             kind="ExternalInput").ap()
    out = nc.dram_tensor("out", [P, NBLK], mybir.dt.float32,
                         kind="ExternalOutput").ap()

    f32 = mybir.dt.float32
    with tile.TileContext(nc) as tc:
        with tc.tile_pool(name="pool", bufs=1) as pool:
            # --- labels first; gathers keep GpSimd busy for ~18us ---
            lab_t = pool.tile([P, J], mybir.dt.int32)
            nc.sync.dma_start(lab_t[:], labels[:])

            c_b = [pool.tile([P, FB], f32, name=f"cb{b}", tag=f"c{b}")
                   for b in range(NBLK)]
            for j in range(J):
                b, jj = divmod(j, JB)
                nc.gpsimd.indirect_dma_start(
                    out=c_b[b][:, jj * D:(jj + 1) * D],
                    out_offset=None,
                    in_=centers[:],
                    in_offset=bass.IndirectOffsetOnAxis(ap=lab_t[:, j:j + 1],
                                                        axis=0),
                )

            # --- x pipeline (overlaps gathers) ---
            x_t = pool.tile([P, F], f32)
            nc.scalar.dma_start(x_t[:], x.rearrange("(p j) d -> p (j d)", p=P))

            xx = pool.tile([P, F], f32)
            nc.vector.tensor_mul(xx[:], x_t[:], x_t[:])
            sx = pool.tile([P, J], f32)
            nc.vector.reduce_sum(sx[:], xx[:].rearrange("p (j d) -> p j d", d=D),
                                 axis=mybir.AxisListType.X)
            # inv = 1 / max(sqrt(sx), EPS)
            mn = pool.tile([P, J], f32)
            nc.scalar.sqrt(mn[:], sx[:])
            nc.vector.tensor_scalar_max(mn[:], mn[:], EPS)
            inv = pool.tile([P, J], f32)
            nc.vector.reciprocal(inv[:], mn[:])
            # nx = x * inv (broadcast inv over each row's D elems)
            iap = inv[:]
            inv_bc = bass.AP(tensor=iap.tensor, offset=iap.offset,
                             ap=list(iap.ap) + [[0, D]])
            nx = pool.tile([P, F], f32)
            nc.vector.tensor_tensor(
                out=nx[:].rearrange("p (j d) -> p j d", d=D),
                in0=x_t[:].rearrange("p (j d) -> p j d", d=D),
                in1=inv_bc,
                op=mybir.AluOpType.mult,
            )

            # --- tail: per block, d = nx - c ; acc[:, b] = sum d^2 ---
            acc = pool.tile([P, NBLK], f32)
            for b in range(NBLK):
                d_b = pool.tile([P, FB], f32, name=f"db{b}", tag=f"d{b}")
                nc.vector.tensor_sub(d_b[:], nx[:, b * FB:(b + 1) * FB],
                                     c_b[b][:])
                nc.vector.tensor_mul(d_b[:], d_b[:], d_b[:])
                nc.vector.reduce_sum(acc[:, b:b + 1], d_b[:],
                                     axis=mybir.AxisListType.X)

            nc.sync.dma_start(out, acc[:])
    nc.compile()
    return nc


def _get_nc():
    if "nc" not in _CACHE:
        _CACHE["nc"] = _build()
    return _CACHE["nc"]


def _run(x, labels, centers, trace=False):
    from concourse.bass_utils import run_bass_kernel_spmd

    x = np.ascontiguousarray(np.asarray(x, dtype=np.float32))
    labels = np.asarray(labels).astype(np.int32)
    centers = np.ascontiguousarray(np.asarray(centers, dtype=np.float32))

    in_maps = []
    for i in range(N_CORES):
        in_maps.append({
            "x": x[i * ROWS:(i + 1) * ROWS],
            "labels": np.ascontiguousarray(
                labels[i * ROWS:(i + 1) * ROWS].reshape(P, J)),
            "centers": centers,
        })
    res = run_bass_kernel_spmd(_get_nc(), in_maps,
                               core_ids=list(range(N_CORES)), trace=trace)
    total = np.float64(0.0)
    for r in res.results:
        total += np.float64(r["out"].sum(dtype=np.float64))
    loss = np.array(np.float32(total / B))
    return loss, res


def kernel(x, labels, centers):
    loss, _ = _run(x, labels, centers, trace=False)
    return loss
